# revision 14
# baseline (speedup 1.0000x reference)
"""Trainium2 Bass kernel for nn_DAC_structure: two-branch patch attention
(softmax(QK^T/sqrt(E)) -> channel mean -> repeat/tile expansion).

Sharding: data-parallel over the outer batch factor (B_outer=8) across the 8
cores; each core owns all 32 channels of one outer-batch element, so the
channel mean is core-local.

v2 layout (memory-regime optimized):
  - host pre-transposes q/k to [(hs,e)=128 partitions, (c,l)] bf16 so no PE
    transposes and half the HBM bytes; per-hp input DMAs so each hp's score
    bank completes as soon as its slice lands.
  - scores: one [K=64,M=32,N=32] (ps) / [K=64,M=8,N=8] (pn) matmul per (c,h)
    into PSUM banks [(hs,chi,l), (clo,s)], tile_position-packed.
  - softmax: ACT exp(0.125*x)->bf16, DVE rowsum/reciprocal/divide.
  - channel mean: 16 accumulating selector matmuls per bank (sum over chi in
    partition space, clo via PSUM accumulation; 1/32 folded into selector).
  - expansion: col-expand via one broadcast copy per bank (PSUM->SBUF bf16);
    row-expand for free in the output DMA via stride-0 (broadcast) source
    dims; outputs written bf16 and upcast to f32 on host.
"""
import sys
import functools

for _p in ('/opt/trn_rl_repo', '/root/.axon_site/_ro/trn_rl_repo'):
    if _p not in sys.path:
        sys.path.append(_p)

import numpy as np
import concourse.bass as bass
import concourse.bacc as bacc
import concourse.tile as tile
from concourse import mybir

F32 = mybir.dt.float32
BF16 = mybir.dt.bfloat16

# problem constants (hardcoded per contract)
CH = 32      # channels per core
L = 32       # patch_size branch seq len (WIN/p)
P8 = 8       # patch_num branch seq len (= patch size)
H = 8        # heads
E = 64       # head dim
WIN = 256
N_CORES = 8
SCALE = 0.125          # 1/sqrt(E)
INV_CH = 1.0 / 32.0    # channel-mean scale, folded into the selectors


def _body(tc: "tile.TileContext", t_pn_d, t_ps_d, out_ps, out_pn):
    nc = tc.nc
    ctx_pools = [tc.tile_pool(name="consts", bufs=1),
                 tc.tile_pool(name="sb", bufs=1),
                 tc.tile_pool(name="psum_bank", bufs=5, space="PSUM"),
                 tc.tile_pool(name="psum_misc", bufs=2, space="PSUM"),
                 tc.tile_pool(name="psum_warm", bufs=1, space="PSUM")]
    consts, sb, psum_bank, psum_misc, psum_warm = (
        p.__enter__() for p in ctx_pools)

    # ---------------- constants ----------------
    # channel-sum selectors (partition-space, stacked diagonals), with the
    # 1/32 channel-mean scale folded in:
    # sel_ps [128, 64]: k=(hs,chi,l) matches m=(hs',l') iff hs==hs', l==l'
    sel_ps = consts.tile([128, 64], BF16, tag="sel_ps")
    nc.gpsimd.memset(sel_ps[:, :], 0.0)
    for chi in range(2):
        nc.gpsimd.affine_select(
            out=sel_ps[:, :], in_=sel_ps[:, :],
            compare_op=mybir.AluOpType.not_equal, fill=INV_CH,
            base=-32 * chi, pattern=[[-64, 2], [-1, 32]], channel_multiplier=1)
    # sel_pn [128, 16]: k=(hs,chigh)*32+i (i<8) matches m=(hs',i')
    sel_pn = consts.tile([128, 16], BF16, tag="sel_pn")
    nc.gpsimd.memset(sel_pn[:, :], 0.0)
    for chigh in range(2):
        nc.gpsimd.affine_select(
            out=sel_pn[:, :], in_=sel_pn[:, :],
            compare_op=mybir.AluOpType.not_equal, fill=INV_CH,
            base=-32 * chigh, pattern=[[-64, 2], [-1, 8]],
            channel_multiplier=1)

    # pn scores land on 8 of every 32 partition rows; zero the bank so the
    # full-tile exp stays finite on the unused rows (selector weights them 0)
    bank_pn = psum_bank.tile([128, 16, 4, 8], F32, tag="bank", name="bank_pn")
    nc.vector.memset(bank_pn[:, :, :, :], 0.0)

    # PE p-state warmers: harmless matmuls on the selector const keep the
    # tensor engine continuously busy (full clock) across input-DMA gaps
    warm = psum_warm.tile([64, 32], F32, tag="warm", name="warm")

    def pe_fill(n):
        for _ in range(n):
            nc.tensor.matmul(warm[:, :], sel_ps[:, :], sel_ps[:, 0:32],
                             start=True, stop=True)

    # ---------------- input DMAs (SP queue, in compute order) -------------
    t_pn = sb.tile([128, 2, 4, 32, 8], BF16, tag="t_pn", name="t_pn")
    nc.sync.dma_start(out=t_pn[:, :, :, :, :], in_=t_pn_d[:, :, :, :, :])
    t_ps = []
    for hp in range(4):
        t = sb.tile([128, 2, 32, 32], BF16, tag=f"t_ps{hp}", name=f"t_ps{hp}")
        nc.sync.dma_start(out=t[:, :, :, :], in_=t_ps_d[hp][:, :, :, :])
        t_ps.append(t)

    # ---------------- PE: scores + selector accumulations -----------------
    def pn_scores():
        for hs in range(2):
            for chigh in range(2):
                pb = hs * 2 + chigh
                for hp in range(4):
                    for clo in range(16):
                        c = chigh * 16 + clo
                        nc.tensor.matmul(
                            bank_pn[pb * 32:pb * 32 + 8, clo, hp, :],
                            t_pn[hs * 64:(hs + 1) * 64, 0, hp, c, :],
                            t_pn[hs * 64:(hs + 1) * 64, 1, hp, c, :],
                            start=True, stop=True,
                            tile_position=(hs * 64, pb * 32))

    banks_ps = [psum_bank.tile([128, 16, 32], F32, tag="bank",
                               name=f"bank_ps{hp}") for hp in range(4)]

    def ps_scores(hp):
        for c in range(CH):
            chi, clo = divmod(c, 16)
            for hs in range(2):
                pb = hs * 2 + chi
                nc.tensor.matmul(
                    banks_ps[hp][pb * 32:(pb + 1) * 32, clo, :],
                    t_ps[hp][hs * 64:(hs + 1) * 64, 0, c, :],
                    t_ps[hp][hs * 64:(hs + 1) * 64, 1, c, :],
                    start=True, stop=True,
                    tile_position=(hs * 64, pb * 32))

    # ---------------- softmax helpers ----------------
    # pn: one full-tile pipeline (junk rows finite, zeroed out by selector)
    p_pn = sb.tile([128, 16, 4, 8], BF16, tag="p_pn", name="p_pn")
    rs_pn = sb.tile([128, 16, 4], BF16, tag="rs_pn", name="rs_pn")
    rr_pn = sb.tile([128, 16, 4], BF16, tag="rr_pn", name="rr_pn")
    pbn = sb.tile([128, 16, 4, 8], BF16, tag="pbn", name="pbn")

    def pn_softmax():
        nc.scalar.activation(out=p_pn[:, :, :, :], in_=bank_pn[:, :, :, :],
                             func=mybir.ActivationFunctionType.Exp,
                             scale=SCALE)
        with nc.allow_low_precision(reason="bf16 rowsum, 2e-2 tolerance"):
            nc.vector.reduce_sum(out=rs_pn[:, :, :], in_=p_pn[:, :, :, :],
                                 axis=mybir.AxisListType.X)
        nc.vector.reciprocal(out=rr_pn[:, :, :], in_=rs_pn[:, :, :])
        nc.vector.tensor_tensor(
            out=pbn[:, :, :, :], in0=p_pn[:, :, :, :],
            in1=rr_pn[:, :, :].unsqueeze(3).broadcast_to([128, 16, 4, 8]),
            op=mybir.AluOpType.mult)

    p_ps = [sb.tile([128, 16, 32], BF16, tag=f"p_ps{hp}", name=f"p_ps{hp}")
            for hp in range(4)]
    rs_ps = [sb.tile([128, 16], BF16, tag=f"rs{hp}", name=f"rs{hp}")
             for hp in range(4)]
    rr_ps = [sb.tile([128, 16], BF16, tag=f"rr{hp}", name=f"rr{hp}")
             for hp in range(4)]
    pb_ps = [sb.tile([128, 16, 32], BF16, tag=f"pb{hp}", name=f"pb{hp}")
             for hp in range(4)]

    def ps_exp(hp):
        nc.scalar.activation(out=p_ps[hp][:, :, :], in_=banks_ps[hp][:, :, :],
                             func=mybir.ActivationFunctionType.Exp,
                             scale=SCALE)

    def ps_norm(hp):
        with nc.allow_low_precision(reason="bf16 rowsum, 2e-2 tolerance"):
            nc.vector.reduce_sum(out=rs_ps[hp][:, :], in_=p_ps[hp][:, :, :],
                                 axis=mybir.AxisListType.X)
        nc.vector.reciprocal(out=rr_ps[hp][:, :], in_=rs_ps[hp][:, :])
        nc.vector.tensor_tensor(
            out=pb_ps[hp][:, :, :], in0=p_ps[hp][:, :, :],
            in1=rr_ps[hp][:, :].unsqueeze(2).broadcast_to([128, 16, 32]),
            op=mybir.AluOpType.mult)

    # ---------------- channel mean (accumulating selector matmuls) --------
    p2n = psum_misc.tile([16, 4, 8], F32, tag="p2", name="p2n")

    def pn_sel():
        for clo in range(16):
            nc.tensor.matmul(p2n[:, :, :], sel_pn[:, :],
                             pbn[:, clo, :, :],
                             start=(clo == 0), stop=(clo == 15))

    p2_ps = [psum_misc.tile([64, 32], F32, tag="p2", name=f"p2_{hp}")
             for hp in range(4)]

    def ps_sel(hp):
        for clo in range(16):
            nc.tensor.matmul(p2_ps[hp][:, :], sel_ps[:, :],
                             pb_ps[hp][:, clo, :],
                             start=(clo == 0), stop=(clo == 15))

    # ---------------- col-expansion (broadcast copies) + output DMAs ------
    # e_pn[p=(hs,i), hp, r, v] = mean_pn[(hs,i), hp, v]  (col tile pattern)
    e_pn = sb.tile([16, 4, 32, 8], BF16, tag="e_pn", name="e_pn")

    def pn_colexp(pair, eng):
        h0 = 2 * pair
        src = p2n[:, h0:h0 + 2, :].unsqueeze(2).broadcast_to([16, 2, 32, 8])
        if eng == 'act':
            nc.scalar.copy(out=e_pn[:, h0:h0 + 2, :, :], in_=src)
        else:
            nc.vector.tensor_copy(out=e_pn[:, h0:h0 + 2, :, :], in_=src)

    # e_ps[hp][p=(hs,l), r, v] = mean_ps[(hs,l), r]  (col repeat pattern)
    e_ps = [sb.tile([64, 32, 8], BF16, tag=f"e_ps{hp}", name=f"e_ps{hp}")
            for hp in range(4)]

    def ps_colexp(hp, eng):
        src = p2_ps[hp][:, :].unsqueeze(2).broadcast_to([64, 32, 8])
        if eng == 'act':
            nc.scalar.copy(out=e_ps[hp][:, :, :], in_=src)
        else:
            nc.vector.tensor_copy(out=e_ps[hp][:, :, :], in_=src)

    def pn_out(hp):
        # out_pn dram layout [hp, (hs,i), blk, col]; blk is a stride-0
        # (broadcast) dim on the SBUF side -> row-expand happens in the DMA.
        # Issued on the Pool SWDGE queue to keep SP's per-DMA overhead off
        # the output phase.
        nc.gpsimd.dma_start(
            out=out_pn[hp, :, :, :],
            in_=e_pn[:, hp, :, :].rearrange("p r v -> p (r v)")
            .unsqueeze(1).broadcast_to([16, 32, 256]))

    def ps_out(hp):
        # out_ps dram layout [hp, (hs,l), rep, col]; rep is broadcast
        nc.sync.dma_start(
            out=out_ps[hp, :, :, :],
            in_=e_ps[hp][:, :, :].rearrange("p r v -> p (r v)")
            .unsqueeze(1).broadcast_to([64, 8, 256]))

    # ---------------- schedule (per-engine program order matters) ---------
    pe_fill(150)      # ramp PE to full clock before pn data lands
    pn_scores()
    pe_fill(45)       # hold clock across the pn->ps0 input-DMA gap
    ps_scores(0)
    with tc.high_priority():
        # the pn tail feeds the first output DMAs; keep the scheduler from
        # deferring it behind ps work
        pn_softmax()
        pn_sel()
        pn_colexp(0, 'act')
        pn_colexp(1, 'vec')
        for hp in range(4):
            pn_out(hp)
    pe_fill(45)
    ps_scores(1)
    ps_exp(0)
    ps_norm(0)
    pe_fill(45)
    ps_scores(2)
    ps_sel(0)
    ps_exp(1)
    ps_norm(1)
    ps_colexp(0, 'act')
    ps_out(0)
    pe_fill(45)
    ps_scores(3)
    ps_sel(1)
    ps_exp(2)
    ps_norm(2)
    ps_colexp(1, 'act')
    ps_out(1)
    ps_sel(2)
    ps_exp(3)
    ps_norm(3)
    ps_colexp(2, 'act')
    ps_out(2)
    ps_sel(3)
    ps_colexp(3, 'vec')
    ps_out(3)

    for p in reversed(ctx_pools):
        p.__exit__(None, None, None)


@functools.lru_cache(maxsize=1)
def _module():
    nc = bacc.Bacc()
    t_pn_d = nc.dram_tensor("in_pn", [128, 2, 4, 32, 8], BF16,
                            kind="ExternalInput")
    t_ps_d = [nc.dram_tensor(f"in_ps{hp}", [128, 2, 32, 32], BF16,
                             kind="ExternalInput") for hp in range(4)]
    # permuted output layouts (host unpermutes): ps [hp, (hs,l), rep, col],
    # pn [hp, (hs,i), blk, col]
    out_ps = nc.dram_tensor("out_ps", [4, 64, 8, 256], BF16,
                            kind="ExternalOutput")
    out_pn = nc.dram_tensor("out_pn", [4, 16, 32, 256], BF16,
                            kind="ExternalOutput")
    with tile.TileContext(nc) as tc, \
            nc.allow_low_precision(reason="bf16 pipeline, 2e-2 tolerance"):
        _body(tc, t_pn_d[:, :, :, :, :],
              [t[:, :, :, :] for t in t_ps_d],
              out_ps[:, :, :, :], out_pn[:, :, :, :])
    nc.compile()
    return nc


def _pack_ps(q, k):
    """[32c, 32l, 8h, 64e] f32 pair -> list of 4 per-hp [128,2,32,32] bf16."""
    import ml_dtypes
    qt = np.transpose(q.reshape(CH, L, 4, 2, E), (3, 4, 2, 0, 1))
    kt = np.transpose(k.reshape(CH, L, 4, 2, E), (3, 4, 2, 0, 1))
    qt = qt.reshape(128, 4, CH, L).astype(ml_dtypes.bfloat16)
    kt = kt.reshape(128, 4, CH, L).astype(ml_dtypes.bfloat16)
    return [np.ascontiguousarray(np.stack([qt[:, hp], kt[:, hp]], axis=1))
            for hp in range(4)]


def _pack_pn(q, k):
    """[32c, 8i, 8h, 64e] f32 pair -> [128, 2, 4, 32, 8] bf16."""
    import ml_dtypes
    qt = np.transpose(q.reshape(CH, P8, 4, 2, E), (3, 4, 2, 0, 1))
    kt = np.transpose(k.reshape(CH, P8, 4, 2, E), (3, 4, 2, 0, 1))
    qt = qt.reshape(128, 4, CH, P8).astype(ml_dtypes.bfloat16)
    kt = kt.reshape(128, 4, CH, P8).astype(ml_dtypes.bfloat16)
    return np.ascontiguousarray(np.stack([qt, kt], axis=1))


def kernel(queries_patch_size, keys_patch_size, queries_patch_num,
           keys_patch_num, patch_index=0, attn_mask=0, **_ignored):
    from concourse.bass_utils import run_bass_kernel_spmd

    q_ps = np.asarray(queries_patch_size, dtype=np.float32)
    k_ps = np.asarray(keys_patch_size, dtype=np.float32)
    q_pn = np.asarray(queries_patch_num, dtype=np.float32)
    k_pn = np.asarray(keys_patch_num, dtype=np.float32)

    nc = _module()
    in_maps = []
    for i in range(N_CORES):
        sl = slice(i * CH, (i + 1) * CH)
        ps = _pack_ps(q_ps[sl], k_ps[sl])
        m = {"in_pn": _pack_pn(q_pn[sl], k_pn[sl])}
        for hp in range(4):
            m[f"in_ps{hp}"] = ps[hp]
        in_maps.append(m)
    res = run_bass_kernel_spmd(nc, in_maps, core_ids=list(range(N_CORES)))

    def unpack_ps(a):
        # [4hp, (hs,l), rep, col] -> [8h, 256, 256]; h=2hp+hs, row=l*8+rep
        return np.asarray(a).astype(np.float32).reshape(8, 256, 256)

    def unpack_pn(a):
        # [4hp, (hs,i), blk, col] -> [8h, 256, 256]; h=2hp+hs, row=blk*8+i
        a = np.asarray(a).astype(np.float32).reshape(4, 2, 8, 32, 256)
        return a.transpose(0, 1, 3, 2, 4).reshape(8, 256, 256)

    s_ps = np.stack([unpack_ps(res.results[i]["out_ps"])
                     for i in range(N_CORES)])
    s_pn = np.stack([unpack_pn(res.results[i]["out_pn"])
                     for i in range(N_CORES)])
    return (s_ps, s_pn)


# revision 15
# speedup vs baseline: 1.0109x; 1.0109x over previous
"""Trainium2 Bass kernel for nn_DAC_structure: two-branch patch attention
(softmax(QK^T/sqrt(E)) -> channel mean -> repeat/tile expansion).

Sharding: data-parallel over the outer batch factor (B_outer=8) across the 8
cores; each core owns all 32 channels of one outer-batch element, so the
channel mean is core-local.

v2 layout (memory-regime optimized):
  - host pre-transposes q/k to [(hs,e)=128 partitions, (c,l)] bf16 so no PE
    transposes and half the HBM bytes; per-hp input DMAs so each hp's score
    bank completes as soon as its slice lands.
  - scores: one [K=64,M=32,N=32] (ps) / [K=64,M=8,N=8] (pn) matmul per (c,h)
    into PSUM banks [(hs,chi,l), (clo,s)], tile_position-packed.
  - softmax: ACT exp(0.125*x)->bf16, DVE rowsum/reciprocal/divide.
  - channel mean: 16 accumulating selector matmuls per bank (sum over chi in
    partition space, clo via PSUM accumulation; 1/32 folded into selector).
  - expansion: col-expand via one broadcast copy per bank (PSUM->SBUF bf16);
    row-expand for free in the output DMA via stride-0 (broadcast) source
    dims; outputs written bf16 and upcast to f32 on host.
"""
import sys
import functools

for _p in ('/opt/trn_rl_repo', '/root/.axon_site/_ro/trn_rl_repo'):
    if _p not in sys.path:
        sys.path.append(_p)

import numpy as np
import concourse.bass as bass
import concourse.bacc as bacc
import concourse.tile as tile
from concourse import mybir

F32 = mybir.dt.float32
BF16 = mybir.dt.bfloat16

# problem constants (hardcoded per contract)
CH = 32      # channels per core
L = 32       # patch_size branch seq len (WIN/p)
P8 = 8       # patch_num branch seq len (= patch size)
H = 8        # heads
E = 64       # head dim
WIN = 256
N_CORES = 8
SCALE = 0.125          # 1/sqrt(E)
INV_CH = 1.0 / 32.0    # channel-mean scale, folded into the selectors


def _body(tc: "tile.TileContext", t_pn_d, t_ps_d, out_ps, out_pn):
    nc = tc.nc
    ctx_pools = [tc.tile_pool(name="consts", bufs=1),
                 tc.tile_pool(name="sb", bufs=1),
                 tc.tile_pool(name="psum_bank", bufs=5, space="PSUM"),
                 tc.tile_pool(name="psum_misc", bufs=2, space="PSUM"),
                 tc.tile_pool(name="psum_warm", bufs=1, space="PSUM")]
    consts, sb, psum_bank, psum_misc, psum_warm = (
        p.__enter__() for p in ctx_pools)

    # ---------------- constants ----------------
    # channel-sum selectors (partition-space, stacked diagonals), with the
    # 1/32 channel-mean scale folded in:
    # sel_ps [128, 64]: k=(hs,chi,l) matches m=(hs',l') iff hs==hs', l==l'
    sel_ps = consts.tile([128, 64], BF16, tag="sel_ps")
    nc.gpsimd.memset(sel_ps[:, :], 0.0)
    for chi in range(2):
        nc.gpsimd.affine_select(
            out=sel_ps[:, :], in_=sel_ps[:, :],
            compare_op=mybir.AluOpType.not_equal, fill=INV_CH,
            base=-32 * chi, pattern=[[-64, 2], [-1, 32]], channel_multiplier=1)
    # sel_pn [128, 16]: k=(hs,chigh)*32+i (i<8) matches m=(hs',i')
    sel_pn = consts.tile([128, 16], BF16, tag="sel_pn")
    nc.gpsimd.memset(sel_pn[:, :], 0.0)
    for chigh in range(2):
        nc.gpsimd.affine_select(
            out=sel_pn[:, :], in_=sel_pn[:, :],
            compare_op=mybir.AluOpType.not_equal, fill=INV_CH,
            base=-32 * chigh, pattern=[[-64, 2], [-1, 8]],
            channel_multiplier=1)

    # pn scores land on 8 of every 32 partition rows; zero the bank so the
    # full-tile exp stays finite on the unused rows (selector weights them 0)
    bank_pn = psum_bank.tile([128, 16, 4, 8], F32, tag="bank", name="bank_pn")
    nc.vector.memset(bank_pn[:, :, :, :], 0.0)

    # PE p-state warmers: harmless matmuls on the selector const keep the
    # tensor engine continuously busy (full clock) across input-DMA gaps
    warm = psum_warm.tile([64, 32], F32, tag="warm", name="warm")

    def pe_fill(n):
        for _ in range(n):
            nc.tensor.matmul(warm[:, :], sel_ps[:, :], sel_ps[:, 0:32],
                             start=True, stop=True)

    # ---------------- input DMAs (SP queue, in compute order) -------------
    t_pn = sb.tile([128, 2, 4, 32, 8], BF16, tag="t_pn", name="t_pn")
    nc.sync.dma_start(out=t_pn[:, :, :, :, :], in_=t_pn_d[:, :, :, :, :])
    t_ps = []
    for hp in range(4):
        t = sb.tile([128, 2, 32, 32], BF16, tag=f"t_ps{hp}", name=f"t_ps{hp}")
        nc.sync.dma_start(out=t[:, :, :, :], in_=t_ps_d[hp][:, :, :, :])
        t_ps.append(t)

    # ---------------- PE: scores + selector accumulations -----------------
    def pn_scores():
        for hs in range(2):
            for chigh in range(2):
                pb = hs * 2 + chigh
                for hp in range(4):
                    for clo in range(16):
                        c = chigh * 16 + clo
                        nc.tensor.matmul(
                            bank_pn[pb * 32:pb * 32 + 8, clo, hp, :],
                            t_pn[hs * 64:(hs + 1) * 64, 0, hp, c, :],
                            t_pn[hs * 64:(hs + 1) * 64, 1, hp, c, :],
                            start=True, stop=True,
                            tile_position=(hs * 64, pb * 32))

    banks_ps = [psum_bank.tile([128, 16, 32], F32, tag="bank",
                               name=f"bank_ps{hp}") for hp in range(4)]

    def ps_scores(hp):
        for c in range(CH):
            chi, clo = divmod(c, 16)
            for hs in range(2):
                pb = hs * 2 + chi
                nc.tensor.matmul(
                    banks_ps[hp][pb * 32:(pb + 1) * 32, clo, :],
                    t_ps[hp][hs * 64:(hs + 1) * 64, 0, c, :],
                    t_ps[hp][hs * 64:(hs + 1) * 64, 1, c, :],
                    start=True, stop=True,
                    tile_position=(hs * 64, pb * 32))

    # ---------------- softmax helpers ----------------
    # pn: one full-tile pipeline (junk rows finite, zeroed out by selector)
    p_pn = sb.tile([128, 16, 4, 8], BF16, tag="p_pn", name="p_pn")
    rs_pn = sb.tile([128, 16, 4], BF16, tag="rs_pn", name="rs_pn")
    rr_pn = sb.tile([128, 16, 4], BF16, tag="rr_pn", name="rr_pn")
    pbn = sb.tile([128, 16, 4, 8], BF16, tag="pbn", name="pbn")

    def pn_softmax():
        nc.scalar.activation(out=p_pn[:, :, :, :], in_=bank_pn[:, :, :, :],
                             func=mybir.ActivationFunctionType.Exp,
                             scale=SCALE)
        with nc.allow_low_precision(reason="bf16 rowsum, 2e-2 tolerance"):
            nc.vector.reduce_sum(out=rs_pn[:, :, :], in_=p_pn[:, :, :, :],
                                 axis=mybir.AxisListType.X)
        nc.vector.reciprocal(out=rr_pn[:, :, :], in_=rs_pn[:, :, :])
        nc.vector.tensor_tensor(
            out=pbn[:, :, :, :], in0=p_pn[:, :, :, :],
            in1=rr_pn[:, :, :].unsqueeze(3).broadcast_to([128, 16, 4, 8]),
            op=mybir.AluOpType.mult)

    p_ps = [sb.tile([128, 16, 32], BF16, tag=f"p_ps{hp}", name=f"p_ps{hp}")
            for hp in range(4)]
    rs_ps = [sb.tile([128, 16], BF16, tag=f"rs{hp}", name=f"rs{hp}")
             for hp in range(4)]
    rr_ps = [sb.tile([128, 16], BF16, tag=f"rr{hp}", name=f"rr{hp}")
             for hp in range(4)]
    pb_ps = [sb.tile([128, 16, 32], BF16, tag=f"pb{hp}", name=f"pb{hp}")
             for hp in range(4)]

    def ps_exp(hp):
        nc.scalar.activation(out=p_ps[hp][:, :, :], in_=banks_ps[hp][:, :, :],
                             func=mybir.ActivationFunctionType.Exp,
                             scale=SCALE)

    def ps_norm(hp):
        with nc.allow_low_precision(reason="bf16 rowsum, 2e-2 tolerance"):
            nc.vector.reduce_sum(out=rs_ps[hp][:, :], in_=p_ps[hp][:, :, :],
                                 axis=mybir.AxisListType.X)
        nc.vector.reciprocal(out=rr_ps[hp][:, :], in_=rs_ps[hp][:, :])
        nc.vector.tensor_tensor(
            out=pb_ps[hp][:, :, :], in0=p_ps[hp][:, :, :],
            in1=rr_ps[hp][:, :].unsqueeze(2).broadcast_to([128, 16, 32]),
            op=mybir.AluOpType.mult)

    # ---------------- channel mean (accumulating selector matmuls) --------
    p2n = psum_misc.tile([16, 4, 8], F32, tag="p2", name="p2n")

    def pn_sel():
        for clo in range(16):
            nc.tensor.matmul(p2n[:, :, :], sel_pn[:, :],
                             pbn[:, clo, :, :],
                             start=(clo == 0), stop=(clo == 15))

    p2_ps = [psum_misc.tile([64, 32], F32, tag="p2", name=f"p2_{hp}")
             for hp in range(4)]

    def ps_sel(hp):
        for clo in range(16):
            nc.tensor.matmul(p2_ps[hp][:, :], sel_ps[:, :],
                             pb_ps[hp][:, clo, :],
                             start=(clo == 0), stop=(clo == 15))

    # ---------------- col-expansion (broadcast copies) + output DMAs ------
    # e_pn[p=(hs,i), hp, r, v] = mean_pn[(hs,i), hp, v]  (col tile pattern)
    e_pn = sb.tile([16, 4, 32, 8], BF16, tag="e_pn", name="e_pn")

    def pn_colexp(pair, eng):
        h0 = 2 * pair
        src = p2n[:, h0:h0 + 2, :].unsqueeze(2).broadcast_to([16, 2, 32, 8])
        if eng == 'act':
            nc.scalar.copy(out=e_pn[:, h0:h0 + 2, :, :], in_=src)
        else:
            nc.vector.tensor_copy(out=e_pn[:, h0:h0 + 2, :, :], in_=src)

    # e_ps[hp][p=(hs,l), r, v] = mean_ps[(hs,l), r]  (col repeat pattern)
    e_ps = [sb.tile([64, 32, 8], BF16, tag=f"e_ps{hp}", name=f"e_ps{hp}")
            for hp in range(4)]

    def ps_colexp(hp, eng):
        src = p2_ps[hp][:, :].unsqueeze(2).broadcast_to([64, 32, 8])
        if eng == 'act':
            nc.scalar.copy(out=e_ps[hp][:, :, :], in_=src)
        else:
            nc.vector.tensor_copy(out=e_ps[hp][:, :, :], in_=src)

    def pn_out(hp):
        # out_pn dram layout [hp, (hs,i), blk, col]; blk is a stride-0
        # (broadcast) dim on the SBUF side -> row-expand happens in the DMA.
        # Issued on the Pool SWDGE queue to keep SP's per-DMA overhead off
        # the output phase.
        nc.gpsimd.dma_start(
            out=out_pn[hp, :, :, :],
            in_=e_pn[:, hp, :, :].rearrange("p r v -> p (r v)")
            .unsqueeze(1).broadcast_to([16, 32, 256]))

    def ps_out(hp):
        # out_ps dram layout [hp, (hs,l), rep, col]; rep is broadcast
        nc.sync.dma_start(
            out=out_ps[hp, :, :, :],
            in_=e_ps[hp][:, :, :].rearrange("p r v -> p (r v)")
            .unsqueeze(1).broadcast_to([64, 8, 256]))

    # ---------------- schedule (per-engine program order matters) ---------
    pe_fill(150)      # ramp PE to full clock before pn data lands
    pn_scores()
    pe_fill(45)       # hold clock across the pn->ps0 input-DMA gap
    ps_scores(0)
    with tc.high_priority():
        # the pn tail feeds the first output DMAs; keep the scheduler from
        # deferring it behind ps work
        pn_softmax()
        pn_sel()
        pn_colexp(0, 'act')
        pn_colexp(1, 'vec')
        for hp in range(4):
            pn_out(hp)
    pe_fill(45)
    ps_scores(1)
    # stamp the ps softmax/output chains with logical ready-times so the
    # tile scheduler doesn't interleave them ahead of the pn chain on DVE/ACT
    with tc.tile_wait_until(0.0075):
        ps_exp(0)
        ps_norm(0)
    pe_fill(45)
    ps_scores(2)
    with tc.tile_wait_until(0.0085):
        ps_sel(0)
        ps_exp(1)
        ps_norm(1)
        ps_colexp(0, 'act')
        ps_out(0)
    pe_fill(45)
    ps_scores(3)
    with tc.tile_wait_until(0.0095):
        ps_sel(1)
        ps_exp(2)
        ps_norm(2)
        ps_colexp(1, 'act')
        ps_out(1)
    with tc.tile_wait_until(0.0105):
        ps_sel(2)
        ps_exp(3)
        ps_norm(3)
        ps_colexp(2, 'act')
        ps_out(2)
        ps_sel(3)
        ps_colexp(3, 'vec')
        ps_out(3)

    for p in reversed(ctx_pools):
        p.__exit__(None, None, None)


@functools.lru_cache(maxsize=1)
def _module():
    nc = bacc.Bacc()
    t_pn_d = nc.dram_tensor("in_pn", [128, 2, 4, 32, 8], BF16,
                            kind="ExternalInput")
    t_ps_d = [nc.dram_tensor(f"in_ps{hp}", [128, 2, 32, 32], BF16,
                             kind="ExternalInput") for hp in range(4)]
    # permuted output layouts (host unpermutes): ps [hp, (hs,l), rep, col],
    # pn [hp, (hs,i), blk, col]
    out_ps = nc.dram_tensor("out_ps", [4, 64, 8, 256], BF16,
                            kind="ExternalOutput")
    out_pn = nc.dram_tensor("out_pn", [4, 16, 32, 256], BF16,
                            kind="ExternalOutput")
    with tile.TileContext(nc) as tc, \
            nc.allow_low_precision(reason="bf16 pipeline, 2e-2 tolerance"):
        _body(tc, t_pn_d[:, :, :, :, :],
              [t[:, :, :, :] for t in t_ps_d],
              out_ps[:, :, :, :], out_pn[:, :, :, :])
    nc.compile()
    return nc


def _pack_ps(q, k):
    """[32c, 32l, 8h, 64e] f32 pair -> list of 4 per-hp [128,2,32,32] bf16."""
    import ml_dtypes
    qt = np.transpose(q.reshape(CH, L, 4, 2, E), (3, 4, 2, 0, 1))
    kt = np.transpose(k.reshape(CH, L, 4, 2, E), (3, 4, 2, 0, 1))
    qt = qt.reshape(128, 4, CH, L).astype(ml_dtypes.bfloat16)
    kt = kt.reshape(128, 4, CH, L).astype(ml_dtypes.bfloat16)
    return [np.ascontiguousarray(np.stack([qt[:, hp], kt[:, hp]], axis=1))
            for hp in range(4)]


def _pack_pn(q, k):
    """[32c, 8i, 8h, 64e] f32 pair -> [128, 2, 4, 32, 8] bf16."""
    import ml_dtypes
    qt = np.transpose(q.reshape(CH, P8, 4, 2, E), (3, 4, 2, 0, 1))
    kt = np.transpose(k.reshape(CH, P8, 4, 2, E), (3, 4, 2, 0, 1))
    qt = qt.reshape(128, 4, CH, P8).astype(ml_dtypes.bfloat16)
    kt = kt.reshape(128, 4, CH, P8).astype(ml_dtypes.bfloat16)
    return np.ascontiguousarray(np.stack([qt, kt], axis=1))


def kernel(queries_patch_size, keys_patch_size, queries_patch_num,
           keys_patch_num, patch_index=0, attn_mask=0, **_ignored):
    from concourse.bass_utils import run_bass_kernel_spmd

    q_ps = np.asarray(queries_patch_size, dtype=np.float32)
    k_ps = np.asarray(keys_patch_size, dtype=np.float32)
    q_pn = np.asarray(queries_patch_num, dtype=np.float32)
    k_pn = np.asarray(keys_patch_num, dtype=np.float32)

    nc = _module()
    in_maps = []
    for i in range(N_CORES):
        sl = slice(i * CH, (i + 1) * CH)
        ps = _pack_ps(q_ps[sl], k_ps[sl])
        m = {"in_pn": _pack_pn(q_pn[sl], k_pn[sl])}
        for hp in range(4):
            m[f"in_ps{hp}"] = ps[hp]
        in_maps.append(m)
    res = run_bass_kernel_spmd(nc, in_maps, core_ids=list(range(N_CORES)))

    def unpack_ps(a):
        # [4hp, (hs,l), rep, col] -> [8h, 256, 256]; h=2hp+hs, row=l*8+rep
        return np.asarray(a).astype(np.float32).reshape(8, 256, 256)

    def unpack_pn(a):
        # [4hp, (hs,i), blk, col] -> [8h, 256, 256]; h=2hp+hs, row=blk*8+i
        a = np.asarray(a).astype(np.float32).reshape(4, 2, 8, 32, 256)
        return a.transpose(0, 1, 3, 2, 4).reshape(8, 256, 256)

    s_ps = np.stack([unpack_ps(res.results[i]["out_ps"])
                     for i in range(N_CORES)])
    s_pn = np.stack([unpack_pn(res.results[i]["out_pn"])
                     for i in range(N_CORES)])
    return (s_ps, s_pn)
